# revision 2
# baseline (speedup 1.0000x reference)
"""Trainium2 Bass kernel for the LoRA-mixture layer.

Math (derived from the reference's interleave):  for batch b,
  y[b] = relu( 0.25 * x[b] @ Bcat_b @ Acat_b )
where Bcat_b = concat of adapter_b[4b:4b+4] along rank (rank 16),
      Acat_b = concat of adapter_a[4b:4b+4] along rank.

Sharding: data-parallel, batch b -> core b (8 batches, 8 cores).

Per-core dataflow (x_i is [4096, 2048] f32):
  for each s-slab of 512 rows:
    DMA in x slab [128p, 4t, 2048d]
    PE-transpose 128x128 blocks -> xT chunks [128d, 512s]  (fp32, exact)
    ACT-evict PSUM->SBUF (rounds to mm dtype)
    mm1: hT[16, 512] += BcatChunk[128,16].T @ xTchunk[128,512]  (16 chunks)
    ACT-evict hT
    mm2: y[128,512] = hTslice[16,128].T @ AcatSlice[16,512]
    DVE relu-evict PSUM->SBUF  (0.25 scale pre-folded into Acat on host)
    DMA out y slab
"""

import numpy as np

import concourse.bass as bass
import concourse.mybir as mybir
import concourse.tile as tile
from concourse import bacc
from concourse.bass_utils import run_bass_kernel_spmd
from concourse.masks import make_identity

B, S, D = 8, 4096, 2048
R = 16               # concatenated rank per batch (4 adapters x rank 4)
N_CORES = 8
SLAB = 512           # s rows per slab
NSLAB = S // SLAB    # 8
TS = SLAB // 128     # 4 s-subtiles per slab
DC = D // 128        # 16 contraction chunks
NDP = D // 512       # 4 output-column chunks

F32 = mybir.dt.float32
F32R = mybir.dt.float32r

MM_DT = F32R         # dtype used for the two matmuls (stationary+moving)


def build_nc():
    nc = bacc.Bacc("TRN2", target_bir_lowering=False, debug=False)

    x = nc.dram_tensor("x", [S, D], F32, kind="ExternalInput")
    bcat = nc.dram_tensor("bcat", [D, R], F32, kind="ExternalInput")
    acat = nc.dram_tensor("acat", [R, D], F32, kind="ExternalInput")
    y = nc.dram_tensor("y", [S, D], F32, kind="ExternalOutput")

    with tile.TileContext(nc) as tc:
        with (
            tc.tile_pool(name="const", bufs=1) as cpool,
            tc.tile_pool(name="xin", bufs=2) as xin_pool,
            tc.tile_pool(name="xt", bufs=20) as xt_pool,
            tc.tile_pool(name="ht", bufs=2) as ht_pool,
            tc.tile_pool(name="yout", bufs=2) as y_pool,
            tc.tile_pool(name="pt", bufs=2, space="PSUM") as pt_pool,
            tc.tile_pool(name="ph", bufs=2, space="PSUM") as ph_pool,
            tc.tile_pool(name="py", bufs=3, space="PSUM") as py_pool,
        ):
            ident = cpool.tile([128, 128], F32)
            make_identity(nc, ident[:])

            # Adapter factors: DMA f32 staging, then round into MM_DT tiles.
            # Bcat [D, R] -> SBUF [128, DC, R]; chunk c = Bcat[c*128:(c+1)*128, :]
            bcat_st = cpool.tile([128, DC, R], F32)
            nc.sync.dma_start(
                out=bcat_st[:], in_=bcat.ap().rearrange("(c p) r -> p c r", p=128)
            )
            # Acat [R, D] (pre-scaled by 0.25 on host)
            acat_st = cpool.tile([R, D], F32)
            nc.sync.dma_start(out=acat_st[:], in_=acat.ap())
            if MM_DT is F32:
                bcat_sb, acat_sb = bcat_st, acat_st
            else:
                bcat_sb = cpool.tile([128, DC, R], MM_DT)
                nc.scalar.copy(bcat_sb[:], bcat_st[:])
                acat_sb = cpool.tile([R, D], MM_DT)
                nc.scalar.copy(acat_sb[:], acat_st[:])

            x_ap = x.ap().rearrange("(i t p) d -> i p t d", p=128, t=TS)
            y_ap = y.ap().rearrange("(i t p) d -> i p t d", p=128, t=TS)

            for i in range(NSLAB):
                x_sb = xin_pool.tile([128, TS, D], F32)
                nc.sync.dma_start(out=x_sb[:], in_=x_ap[i])

                # transpose x slab into DC chunks of [128 d, SLAB s]
                xt_chunks = []
                for c in range(DC):
                    pt = pt_pool.tile([128, SLAB], F32, tag="pt")
                    for t in range(TS):
                        nc.tensor.transpose(
                            pt[:, t * 128 : (t + 1) * 128],
                            x_sb[:, t, c * 128 : (c + 1) * 128],
                            ident[:],
                        )
                    xt_sb = xt_pool.tile([128, SLAB], MM_DT, tag="xt")
                    nc.scalar.copy(xt_sb[:], pt[:])
                    xt_chunks.append(xt_sb)

                # mm1: hT [R, SLAB] accumulated over DC chunks
                ht_ps = ph_pool.tile([R, SLAB], F32, tag="ph")
                for c in range(DC):
                    nc.tensor.matmul(
                        ht_ps[:],
                        bcat_sb[:, c, :],
                        xt_chunks[c][:],
                        start=(c == 0),
                        stop=(c == DC - 1),
                    )
                ht_sb = ht_pool.tile([R, SLAB], MM_DT, tag="ht")
                nc.scalar.copy(ht_sb[:], ht_ps[:])

                # mm2 + relu eviction
                y_sb = y_pool.tile([128, TS, D], F32)
                for t in range(TS):
                    for dp in range(NDP):
                        py = py_pool.tile([128, 512], F32, tag="py")
                        nc.tensor.matmul(
                            py[:],
                            ht_sb[:, t * 128 : (t + 1) * 128],
                            acat_sb[:, dp * 512 : (dp + 1) * 512],
                            start=True,
                            stop=True,
                        )
                        nc.vector.tensor_scalar_max(
                            y_sb[:, t, dp * 512 : (dp + 1) * 512], py[:], 0.0
                        )
                nc.sync.dma_start(out=y_ap[i], in_=y_sb[:])

    nc.compile()
    return nc


_NC = None


def _get_nc():
    global _NC
    if _NC is None:
        _NC = build_nc()
    return _NC


def make_in_maps(x, adapter_b, adapter_a):
    in_maps = []
    for b in range(B):
        bc = np.ascontiguousarray(
            adapter_b[4 * b : 4 * b + 4].transpose(1, 0, 2).reshape(D, R)
        ).astype(np.float32)
        ac = np.ascontiguousarray(
            adapter_a[4 * b : 4 * b + 4].reshape(R, D) * 0.25
        ).astype(np.float32)
        in_maps.append(
            {
                "x": np.ascontiguousarray(x[b]).astype(np.float32),
                "bcat": bc,
                "acat": ac,
            }
        )
    return in_maps


def run(x, adapter_b, adapter_a, **run_kwargs):
    nc = _get_nc()
    in_maps = make_in_maps(x, adapter_b, adapter_a)
    res = run_bass_kernel_spmd(nc, in_maps, list(range(N_CORES)), **run_kwargs)
    out = np.stack([res.results[i]["y"] for i in range(N_CORES)])
    return out, res


def kernel(x, adapter_b, adapter_a):
    out, _ = run(x, adapter_b, adapter_a)
    return out
